# revision 17
# baseline (speedup 1.0000x reference)
"""Trainium2 Bass kernel for a single-step vanilla RNN cell + projection + softmax.

Math (per row of the batch):
    h_new = tanh(x @ W_ih.T + b_ih + h @ W_hh.T + b_hh)
    probs = softmax(h_new @ W_proj.T + b_proj, axis=-1)
Returns (probs, h_new) as float32, matching the reference.

Strategy: pure data parallel over the batch dim n across 8 NeuronCores.
The host pre-transposes/casts activations to bf16 column-layout (features
on partitions, batch on the free dim), so the device runs a transpose-free
matmul pipeline:
  per 512-row chunk:
    h_newT[128,512]x2 (PSUM f32) = W_ihT_aug.T @ x_augT + W_hhT.T @ hT
       (bias rows folded into the augmented x/W_ih matrices)
    tanh -> bf16 SBUF tile (doubles as projection lhsT and DMA-out source)
    logits[128rows,20] = h_newT_slice.T @ W_projT    (row layout)
    softmax via exp (ACT) * exp(b_proj) const, row-sum, reciprocal, scale
Outputs: h_newT bf16 [256, n_c] (host transposes back to [n,256] f32),
probs f32 [n_c, 20] directly in natural layout.
"""

import os

import numpy as np
import ml_dtypes

import concourse.bass as bass
import concourse.mybir as mybir
import concourse.tile as tile
from concourse.bass_utils import run_bass_kernel_spmd

BF16 = mybir.dt.bfloat16
F32 = mybir.dt.float32
bf16 = ml_dtypes.bfloat16

N_CORES = 8
HID = 256
VIN = 40
VOUT = 20
CHUNK = 512
RB = CHUNK // 128  # row blocks of 128 per chunk

# Exposed for test harnesses: the BassKernelResults of the last run.
LAST_RESULT = None


SUPER = 4096  # rows per DMA superchunk (1 MB bf16 hidden-state transfers)
QRB = SUPER // 128  # row blocks of 128 per superchunk


def build_nc(n_c: int, dma: str = "sync", split_waits: bool = True) -> bass.Bass:
    assert n_c % SUPER == 0
    nsuper = n_c // SUPER
    nsub = SUPER // CHUNK
    nc = bass.Bass()
    dma_eng = {"sync": lambda: nc.sync, "gpsimd": lambda: nc.gpsimd}[dma]()
    # loads on the SP HWDGE ring, stores on the ACT HWDGE ring: stores wait on
    # ACT-produced data (same-engine program order, no blocking), so prefetch
    # loads are never queued behind a store that is waiting for compute.
    st_eng = nc.scalar if dma == "sync" else dma_eng

    xat_d = nc.dram_tensor("xat", [VIN + 1, n_c], BF16, kind="ExternalInput")
    ht_d = nc.dram_tensor("ht", [HID, n_c], BF16, kind="ExternalInput")
    w1_d = nc.dram_tensor("w1", [VIN + 1, HID], BF16, kind="ExternalInput")
    whh_d = nc.dram_tensor("whh", [HID, HID], BF16, kind="ExternalInput")
    wp_d = nc.dram_tensor("wp", [HID, VOUT], BF16, kind="ExternalInput")
    ebp_d = nc.dram_tensor("ebp", [128, RB, VOUT], F32, kind="ExternalInput")
    hnt_d = nc.dram_tensor("hnt", [HID, n_c], BF16, kind="ExternalOutput")
    # probs are stored in the SBUF-native interleaved layout (one contiguous
    # 2.5KB run per partition per superchunk); the host de-interleaves.
    pr_d = nc.dram_tensor(
        "probs", [n_c // SUPER, 128, QRB, VOUT], F32, kind="ExternalOutput"
    )

    with tile.TileContext(nc) as tc:
        with (
            tc.tile_pool(name="consts", bufs=1) as consts,
            tc.tile_pool(name="io", bufs=3) as io,
            tc.tile_pool(name="hsb", bufs=2) as hsb,
            tc.tile_pool(name="prs", bufs=2) as prs,
            tc.tile_pool(name="sfx", bufs=3) as sfx,
            tc.tile_pool(name="ps_h", bufs=4, space="PSUM") as ps_h,
            tc.tile_pool(name="ps_l", bufs=4, space="PSUM") as ps_l,
        ):
            w1 = consts.tile([VIN + 1, HID], BF16)
            nc.gpsimd.dma_start(out=w1[:], in_=w1_d[:])
            whh0 = consts.tile([128, HID], BF16, tag="whh0")
            whh1 = consts.tile([128, HID], BF16, tag="whh1")
            nc.gpsimd.dma_start(out=whh0[:], in_=whh_d[0:128, :])
            nc.gpsimd.dma_start(out=whh1[:], in_=whh_d[128:256, :])
            wp0 = consts.tile([128, VOUT], BF16, tag="wp0")
            wp1 = consts.tile([128, VOUT], BF16, tag="wp1")
            nc.gpsimd.dma_start(out=wp0[:], in_=wp_d[0:128, :])
            nc.gpsimd.dma_start(out=wp1[:], in_=wp_d[128:256, :])
            ebp = consts.tile([128, RB, VOUT], F32)
            nc.gpsimd.dma_start(out=ebp[:], in_=ebp_d[:])

            # Per-superchunk live tiles, keyed sc % 2 (pools have bufs>=2).
            tiles = {}

            def emit_loads(sc):
                ss = sc * SUPER
                xat = io.tile([VIN + 1, SUPER], BF16, tag="xat")
                ht0 = io.tile([128, SUPER], BF16, tag="ht0")
                ht1 = io.tile([128, SUPER], BF16, tag="ht1")
                if sc == 0:
                    # strip-load the first superchunk so the first matmuls can
                    # start after one 512-column strip instead of a full 1MB
                    for q in range(nsub):
                        qsl = slice(q * CHUNK, (q + 1) * CHUNK)
                        dma_eng.dma_start(out=xat[:, qsl], in_=xat_d[:, qsl])
                        dma_eng.dma_start(out=ht0[:, qsl], in_=ht_d[0:128, qsl])
                        dma_eng.dma_start(out=ht1[:, qsl], in_=ht_d[128:256, qsl])
                else:
                    dma_eng.dma_start(out=xat[:], in_=xat_d[:, ss : ss + SUPER])
                    dma_eng.dma_start(out=ht0[:], in_=ht_d[0:128, ss : ss + SUPER])
                    dma_eng.dma_start(out=ht1[:], in_=ht_d[128:256, ss : ss + SUPER])
                hnt0 = hsb.tile([128, SUPER], BF16, tag="hnt0")
                hnt1 = hsb.tile([128, SUPER], BF16, tag="hnt1")
                pr = prs.tile([128, QRB, VOUT], F32, tag="pr")
                tiles[sc % 2] = (xat, ht0, ht1, hnt0, hnt1, pr)

            def emit_mm(sc, q):
                xat, ht0, ht1, _, _, _ = tiles[sc % 2]
                qs = q * CHUNK
                out = []
                for m in range(2):
                    ms = m * 128
                    ps = ps_h.tile([128, CHUNK], F32, tag="hnt_ps")
                    nc.tensor.matmul(
                        ps[:], w1[:, ms : ms + 128], xat[:, qs : qs + CHUNK],
                        start=True, stop=False,
                    )
                    nc.tensor.matmul(
                        ps[:], whh0[:, ms : ms + 128], ht0[:, qs : qs + CHUNK],
                        start=False, stop=False,
                    )
                    nc.tensor.matmul(
                        ps[:], whh1[:, ms : ms + 128], ht1[:, qs : qs + CHUNK],
                        start=False, stop=True,
                    )
                    out.append(ps)
                return out

            def emit_tanh(sc, q, pss):
                _, _, _, hnt0, hnt1, _ = tiles[sc % 2]
                qs = q * CHUNK
                for m, hnt in enumerate((hnt0, hnt1)):
                    nc.scalar.activation(
                        hnt[:, qs : qs + CHUNK], pss[m][:],
                        mybir.ActivationFunctionType.Tanh,
                    )

            def emit_proj(sc, q):
                _, _, _, hnt0, hnt1, pr = tiles[sc % 2]
                qs = q * CHUNK
                # one PSUM bank per row-block (pool rotation) so consecutive
                # accumulation groups never serialize on the same bank
                lgs = []
                for r in range(RB):
                    rs = qs + r * 128
                    lg = ps_l.tile([128, VOUT], F32, tag="lg")
                    nc.tensor.matmul(
                        lg[:], hnt0[:, rs : rs + 128], wp0[:],
                        start=True, stop=False,
                    )
                    nc.tensor.matmul(
                        lg[:], hnt1[:, rs : rs + 128], wp1[:],
                        start=False, stop=True,
                    )
                    lgs.append(lg)
                et = sfx.tile([128, RB, VOUT], F32, tag="et")
                for r in range(RB):
                    nc.scalar.activation(
                        et[:, r, :], lgs[r][:], mybir.ActivationFunctionType.Exp
                    )
                u = sfx.tile([128, RB, VOUT], F32, tag="u")
                nc.vector.tensor_mul(u[:], et[:], ebp[:])
                sm = sfx.tile([128, RB], F32, tag="sm")
                nc.vector.reduce_sum(sm[:], u[:], axis=mybir.AxisListType.X)
                rc = sfx.tile([128, RB], F32, tag="rc")
                nc.vector.reciprocal(rc[:], sm[:])
                for r in range(RB):
                    nc.vector.tensor_scalar_mul(
                        pr[:, q * RB + r, :], u[:, r, :], rc[:, r : r + 1]
                    )

            def emit_stores(sc):
                ss = sc * SUPER
                _, _, _, hnt0, hnt1, pr = tiles[sc % 2]
                st_eng.dma_start(out=hnt_d[0:128, ss : ss + SUPER], in_=hnt0[:])
                st_eng.dma_start(out=hnt_d[128:256, ss : ss + SUPER], in_=hnt1[:])
                st_eng.dma_start(out=pr_d[sc], in_=pr[:])

            # One-chunk software pipeline: PE stream is [MM g][proj g-1] so the
            # projection never waits on the tanh of its own chunk; tanh(g) is
            # emitted after proj(g-1) to keep it off PE's critical path.
            nchunks = nsuper * nsub
            prev = None
            for g in range(nchunks):
                sc, q = divmod(g, nsub)
                if q == 0:
                    emit_loads(sc)
                pss = emit_mm(sc, q)
                if prev is not None:
                    emit_proj(*prev)
                    psc, pq = prev
                    if pq == nsub - 1:
                        emit_stores(psc)
                emit_tanh(sc, q, pss)
                prev = (sc, q)
            emit_proj(*prev)
            emit_stores(prev[0])
    if split_waits:
        _split_excess_waits(nc)
    return nc


def _split_excess_waits(nc: bass.Bass) -> None:
    """Walrus allows only one sync-wait command per lowered instruction (the
    64B EVENTS struct has a single wait slot), but Tile's sem-assignment can
    attach 2+ waits (e.g. a data dep + the HWDGE queue-head wait) because the
    redundant-wait optimizer is disabled. Splitting the extra waits into
    standalone event-semaphore instructions immediately before, on the same
    engine stream, is semantically identical: the issuing sequencer blocks on
    them in program order before dispatching the instruction."""
    skip = {"InstCall", "InstUnconditionalBranch", "InstISA"}
    for fn in nc.m.functions:
        for bb in fn.blocks:
            out = []
            changed = False
            for ins in bb.instructions:
                si = ins.sync_info
                if (
                    si is not None
                    and len(si.on_wait) > 1
                    and type(ins).__name__ not in skip
                ):
                    waits = list(si.on_wait)
                    keep_idx = len(waits) - 1
                    for i, w in enumerate(waits):
                        if w.ant_name and w.ant_name.startswith(("DMAHW", "DMASW")):
                            keep_idx = i
                            break
                    for i, w in enumerate(waits):
                        if i == keep_idx:
                            continue
                        out.append(
                            mybir.InstEventSemaphore(
                                name=f"{ins.name}-xw{i}",
                                engine=ins.engine,
                                sync_info=mybir.SyncInfo(on_wait=[w], on_update=[]),
                                debug=ins.debug,
                            )
                        )
                    ins.sync_info = mybir.SyncInfo(
                        on_wait=[waits[keep_idx]], on_update=list(si.on_update)
                    )
                    changed = True
                out.append(ins)
            if changed:
                bb.instructions = out


_NC_CACHE = {}


def _get_nc(n_c: int) -> bass.Bass:
    if n_c not in _NC_CACHE:
        _NC_CACHE[n_c] = build_nc(n_c)
    return _NC_CACHE[n_c]


def host_inputs(x, h, W_ih, W_hh, b_ih, b_hh, W_proj, b_proj, n_cores=N_CORES):
    """Shard + relayout the full f32 inputs into per-core bf16 in_maps."""
    x = np.asarray(x, dtype=np.float32)
    h = np.asarray(h, dtype=np.float32)
    n = x.shape[0]
    n_c = n // n_cores

    w1 = np.empty((VIN + 1, HID), dtype=bf16)
    w1[:VIN] = np.asarray(W_ih, np.float32).T.astype(bf16)
    w1[VIN] = (
        np.asarray(b_ih, np.float32) + np.asarray(b_hh, np.float32)
    ).astype(bf16)
    whh = np.asarray(W_hh, np.float32).T.astype(bf16)
    wp = np.asarray(W_proj, np.float32).T.astype(bf16)
    ebp = np.ascontiguousarray(
        np.broadcast_to(
            np.exp(np.asarray(b_proj, np.float32))[None, None, :], (128, RB, VOUT)
        ),
        dtype=np.float32,
    )

    in_maps = []
    for i in range(n_cores):
        sl = slice(i * n_c, (i + 1) * n_c)
        xat = np.empty((VIN + 1, n_c), dtype=bf16)
        xat[:VIN] = x[sl].T.astype(bf16)
        xat[VIN] = 1.0
        ht = h[sl].T.astype(bf16)
        in_maps.append(
            {"xat": xat, "ht": ht, "w1": w1, "whh": whh, "wp": wp, "ebp": ebp}
        )
    return in_maps, n_c


def kernel(x, h, W_ih, W_hh, b_ih, b_hh, W_proj, b_proj):
    global LAST_RESULT
    in_maps, n_c = host_inputs(x, h, W_ih, W_hh, b_ih, b_hh, W_proj, b_proj)
    nc = _get_nc(n_c)
    trace = bool(os.environ.get("KERNEL_TRACE"))
    res = run_bass_kernel_spmd(nc, in_maps, list(range(N_CORES)), trace=trace)
    LAST_RESULT = res

    n = n_c * N_CORES
    probs = np.empty((n, VOUT), np.float32)
    h_new = np.empty((n, HID), np.float32)
    for i in range(N_CORES):
        sl = slice(i * n_c, (i + 1) * n_c)
        # de-interleave [nsuper, 128, QRB, VOUT] -> rows (sc, rb, p) order
        probs[sl] = (
            res.results[i]["probs"].transpose(0, 2, 1, 3).reshape(n_c, VOUT)
        )
        h_new[sl] = res.results[i]["hnt"].T.astype(np.float32)
    return probs, h_new


# revision 20
# speedup vs baseline: 1.1105x; 1.1105x over previous
"""Trainium2 Bass kernel for a single-step vanilla RNN cell + projection + softmax.

Math (per row of the batch):
    h_new = tanh(x @ W_ih.T + b_ih + h @ W_hh.T + b_hh)
    probs = softmax(h_new @ W_proj.T + b_proj, axis=-1)
Returns (probs, h_new) as float32, matching the reference.

Strategy: pure data parallel over the batch dim n across 8 NeuronCores.
The host pre-transposes/casts activations to bf16 column-layout (features
on partitions, batch on the free dim), so the device runs a transpose-free
matmul pipeline:
  per 512-row chunk:
    h_newT[128,512]x2 (PSUM f32) = W_ihT_aug.T @ x_augT + W_hhT.T @ hT
       (bias rows folded into the augmented x/W_ih matrices)
    tanh -> bf16 SBUF tile (doubles as projection lhsT and DMA-out source)
    logits[128rows,20] = h_newT_slice.T @ W_projT    (row layout)
    softmax via exp (ACT) * exp(b_proj) const, row-sum, reciprocal, scale
Outputs: h_newT bf16 [256, n_c] (host transposes back to [n,256] f32),
probs f32 [n_c, 20] directly in natural layout.
"""

import os

import numpy as np
import ml_dtypes

import concourse.bass as bass
import concourse.mybir as mybir
import concourse.tile as tile
from concourse.bass_utils import run_bass_kernel_spmd

BF16 = mybir.dt.bfloat16
F32 = mybir.dt.float32
bf16 = ml_dtypes.bfloat16

N_CORES = 8
HID = 256
VIN = 40
VOUT = 20
CHUNK = 512
RB = CHUNK // 128  # row blocks of 128 per chunk

# Exposed for test harnesses: the BassKernelResults of the last run.
LAST_RESULT = None


SUPER = 4096  # rows per DMA superchunk (1 MB bf16 hidden-state transfers)
QRB = SUPER // 128  # row blocks of 128 per superchunk


def build_nc(n_c: int, dma: str = "sync", split_waits: bool = True) -> bass.Bass:
    assert n_c % SUPER == 0
    nsuper = n_c // SUPER
    nsub = SUPER // CHUNK
    nc = bass.Bass()
    dma_eng = {"sync": lambda: nc.sync, "gpsimd": lambda: nc.gpsimd}[dma]()
    # loads on the SP HWDGE ring, stores on the ACT HWDGE ring: stores wait on
    # ACT-produced data (same-engine program order, no blocking), so prefetch
    # loads are never queued behind a store that is waiting for compute.
    st_eng = nc.gpsimd

    xat_d = nc.dram_tensor("xat", [VIN + 1, n_c], BF16, kind="ExternalInput")
    ht_d = nc.dram_tensor("ht", [HID, n_c], BF16, kind="ExternalInput")
    w1_d = nc.dram_tensor("w1", [VIN + 1, HID], BF16, kind="ExternalInput")
    whh_d = nc.dram_tensor("whh", [HID, HID], BF16, kind="ExternalInput")
    wp_d = nc.dram_tensor("wp", [HID, VOUT], BF16, kind="ExternalInput")
    ebp_d = nc.dram_tensor("ebp", [128, RB, VOUT], F32, kind="ExternalInput")
    hnt_d = nc.dram_tensor("hnt", [HID, n_c], BF16, kind="ExternalOutput")
    # probs are stored in the SBUF-native interleaved layout (one contiguous
    # 2.5KB run per partition per superchunk); the host de-interleaves.
    pr_d = nc.dram_tensor(
        "probs", [n_c // SUPER, 128, QRB, VOUT], F32, kind="ExternalOutput"
    )

    with tile.TileContext(nc) as tc:
        with (
            tc.tile_pool(name="consts", bufs=1) as consts,
            tc.tile_pool(name="io", bufs=3) as io,
            tc.tile_pool(name="hsb", bufs=2) as hsb,
            tc.tile_pool(name="prs", bufs=2) as prs,
            tc.tile_pool(name="sfx", bufs=3) as sfx,
            tc.tile_pool(name="ps_h", bufs=4, space="PSUM") as ps_h,
            tc.tile_pool(name="ps_l", bufs=2, space="PSUM") as ps_l,
        ):
            w1 = consts.tile([VIN + 1, HID], BF16)
            nc.gpsimd.dma_start(out=w1[:], in_=w1_d[:])
            whh0 = consts.tile([128, HID], BF16, tag="whh0")
            whh1 = consts.tile([128, HID], BF16, tag="whh1")
            nc.gpsimd.dma_start(out=whh0[:], in_=whh_d[0:128, :])
            nc.gpsimd.dma_start(out=whh1[:], in_=whh_d[128:256, :])
            wp0 = consts.tile([128, VOUT], BF16, tag="wp0")
            wp1 = consts.tile([128, VOUT], BF16, tag="wp1")
            nc.gpsimd.dma_start(out=wp0[:], in_=wp_d[0:128, :])
            nc.gpsimd.dma_start(out=wp1[:], in_=wp_d[128:256, :])
            ebp = consts.tile([128, RB, VOUT], F32)
            nc.gpsimd.dma_start(out=ebp[:], in_=ebp_d[:])

            # Per-superchunk live tiles, keyed sc % 2 (pools have bufs>=2).
            tiles = {}

            def emit_loads(sc):
                ss = sc * SUPER
                xat = io.tile([VIN + 1, SUPER], BF16, tag="xat")
                ht0 = io.tile([128, SUPER], BF16, tag="ht0")
                ht1 = io.tile([128, SUPER], BF16, tag="ht1")
                if sc == 0:
                    # strip-load the first superchunk so the first matmuls can
                    # start after one 512-column strip instead of a full 1MB
                    for q in range(nsub):
                        qsl = slice(q * CHUNK, (q + 1) * CHUNK)
                        dma_eng.dma_start(out=xat[:, qsl], in_=xat_d[:, qsl])
                        dma_eng.dma_start(out=ht0[:, qsl], in_=ht_d[0:128, qsl])
                        dma_eng.dma_start(out=ht1[:, qsl], in_=ht_d[128:256, qsl])
                else:
                    dma_eng.dma_start(out=xat[:], in_=xat_d[:, ss : ss + SUPER])
                    dma_eng.dma_start(out=ht0[:], in_=ht_d[0:128, ss : ss + SUPER])
                    dma_eng.dma_start(out=ht1[:], in_=ht_d[128:256, ss : ss + SUPER])
                hnt0 = hsb.tile([128, SUPER], BF16, tag="hnt0")
                hnt1 = hsb.tile([128, SUPER], BF16, tag="hnt1")
                pr = prs.tile([128, QRB, VOUT], F32, tag="pr")
                tiles[sc % 2] = (xat, ht0, ht1, hnt0, hnt1, pr)

            def emit_mm(sc, q):
                xat, ht0, ht1, _, _, _ = tiles[sc % 2]
                qs = q * CHUNK
                out = []
                for m in range(2):
                    ms = m * 128
                    ps = ps_h.tile([128, CHUNK], F32, tag="hnt_ps")
                    nc.tensor.matmul(
                        ps[:], w1[:, ms : ms + 128], xat[:, qs : qs + CHUNK],
                        start=True, stop=False,
                    )
                    nc.tensor.matmul(
                        ps[:], whh0[:, ms : ms + 128], ht0[:, qs : qs + CHUNK],
                        start=False, stop=False,
                    )
                    nc.tensor.matmul(
                        ps[:], whh1[:, ms : ms + 128], ht1[:, qs : qs + CHUNK],
                        start=False, stop=True,
                    )
                    out.append(ps)
                return out

            def emit_tanh(sc, q, pss):
                _, _, _, hnt0, hnt1, _ = tiles[sc % 2]
                qs = q * CHUNK
                for m, hnt in enumerate((hnt0, hnt1)):
                    nc.scalar.activation(
                        hnt[:, qs : qs + CHUNK], pss[m][:],
                        mybir.ActivationFunctionType.Tanh,
                    )

            def emit_proj_mm(sc, q):
                _, _, _, hnt0, hnt1, _ = tiles[sc % 2]
                qs = q * CHUNK
                lg = ps_l.tile([128, RB, VOUT], F32, tag="lg")
                for r in range(RB):
                    rs = qs + r * 128
                    nc.tensor.matmul(
                        lg[:, r, :], hnt0[:, rs : rs + 128], wp0[:],
                        start=True, stop=False,
                    )
                    nc.tensor.matmul(
                        lg[:, r, :], hnt1[:, rs : rs + 128], wp1[:],
                        start=False, stop=True,
                    )
                return lg

            def emit_softmax(sc, q, lg):
                _, _, _, _, _, pr = tiles[sc % 2]
                et = sfx.tile([128, RB, VOUT], F32, tag="et")
                nc.scalar.activation(et[:], lg[:], mybir.ActivationFunctionType.Exp)
                u = sfx.tile([128, RB, VOUT], F32, tag="u")
                nc.vector.tensor_mul(u[:], et[:], ebp[:])
                sm = sfx.tile([128, RB], F32, tag="sm")
                nc.vector.reduce_sum(sm[:], u[:], axis=mybir.AxisListType.X)
                rc = sfx.tile([128, RB], F32, tag="rc")
                nc.vector.reciprocal(rc[:], sm[:])
                for r in range(RB):
                    nc.vector.tensor_scalar_mul(
                        pr[:, q * RB + r, :], u[:, r, :], rc[:, r : r + 1]
                    )

            def emit_stores(sc):
                ss = sc * SUPER
                _, _, _, hnt0, hnt1, pr = tiles[sc % 2]
                st_eng.dma_start(out=hnt_d[0:128, ss : ss + SUPER], in_=hnt0[:])
                st_eng.dma_start(out=hnt_d[128:256, ss : ss + SUPER], in_=hnt1[:])
                st_eng.dma_start(out=pr_d[sc], in_=pr[:])

            # One-chunk software pipeline. Per iteration the PE stream is
            # [MM g][proj g-1] so the projection never waits on its own tanh,
            # and the ACT stream is [tanh g][exp g-1] so the next projection's
            # tanh is never queued behind softmax work.
            nchunks = nsuper * nsub
            prev = None
            for g in range(nchunks):
                sc, q = divmod(g, nsub)
                if q == 0:
                    emit_loads(sc)
                pss = emit_mm(sc, q)
                plg = emit_proj_mm(*prev) if prev is not None else None
                emit_tanh(sc, q, pss)
                if prev is not None:
                    emit_softmax(*prev, plg)
                    psc, pq = prev
                    if pq == nsub - 1:
                        emit_stores(psc)
                prev = (sc, q)
            plg = emit_proj_mm(*prev)
            emit_softmax(*prev, plg)
            emit_stores(prev[0])
    if split_waits:
        _split_excess_waits(nc)
    return nc


def _split_excess_waits(nc: bass.Bass) -> None:
    """Walrus allows only one sync-wait command per lowered instruction (the
    64B EVENTS struct has a single wait slot), but Tile's sem-assignment can
    attach 2+ waits (e.g. a data dep + the HWDGE queue-head wait) because the
    redundant-wait optimizer is disabled. Splitting the extra waits into
    standalone event-semaphore instructions immediately before, on the same
    engine stream, is semantically identical: the issuing sequencer blocks on
    them in program order before dispatching the instruction."""
    skip = {"InstCall", "InstUnconditionalBranch", "InstISA"}
    for fn in nc.m.functions:
        for bb in fn.blocks:
            out = []
            changed = False
            for ins in bb.instructions:
                si = ins.sync_info
                if (
                    si is not None
                    and len(si.on_wait) > 1
                    and type(ins).__name__ not in skip
                ):
                    waits = list(si.on_wait)
                    keep_idx = len(waits) - 1
                    for i, w in enumerate(waits):
                        if w.ant_name and w.ant_name.startswith(("DMAHW", "DMASW")):
                            keep_idx = i
                            break
                    for i, w in enumerate(waits):
                        if i == keep_idx:
                            continue
                        out.append(
                            mybir.InstEventSemaphore(
                                name=f"{ins.name}-xw{i}",
                                engine=ins.engine,
                                sync_info=mybir.SyncInfo(on_wait=[w], on_update=[]),
                                debug=ins.debug,
                            )
                        )
                    ins.sync_info = mybir.SyncInfo(
                        on_wait=[waits[keep_idx]], on_update=list(si.on_update)
                    )
                    changed = True
                out.append(ins)
            if changed:
                bb.instructions = out


_NC_CACHE = {}


def _get_nc(n_c: int) -> bass.Bass:
    if n_c not in _NC_CACHE:
        _NC_CACHE[n_c] = build_nc(n_c)
    return _NC_CACHE[n_c]


def host_inputs(x, h, W_ih, W_hh, b_ih, b_hh, W_proj, b_proj, n_cores=N_CORES):
    """Shard + relayout the full f32 inputs into per-core bf16 in_maps."""
    x = np.asarray(x, dtype=np.float32)
    h = np.asarray(h, dtype=np.float32)
    n = x.shape[0]
    n_c = n // n_cores

    w1 = np.empty((VIN + 1, HID), dtype=bf16)
    w1[:VIN] = np.asarray(W_ih, np.float32).T.astype(bf16)
    w1[VIN] = (
        np.asarray(b_ih, np.float32) + np.asarray(b_hh, np.float32)
    ).astype(bf16)
    whh = np.asarray(W_hh, np.float32).T.astype(bf16)
    wp = np.asarray(W_proj, np.float32).T.astype(bf16)
    ebp = np.ascontiguousarray(
        np.broadcast_to(
            np.exp(np.asarray(b_proj, np.float32))[None, None, :], (128, RB, VOUT)
        ),
        dtype=np.float32,
    )

    in_maps = []
    for i in range(n_cores):
        sl = slice(i * n_c, (i + 1) * n_c)
        xat = np.empty((VIN + 1, n_c), dtype=bf16)
        xat[:VIN] = x[sl].T.astype(bf16)
        xat[VIN] = 1.0
        ht = h[sl].T.astype(bf16)
        in_maps.append(
            {"xat": xat, "ht": ht, "w1": w1, "whh": whh, "wp": wp, "ebp": ebp}
        )
    return in_maps, n_c


def kernel(x, h, W_ih, W_hh, b_ih, b_hh, W_proj, b_proj):
    global LAST_RESULT
    in_maps, n_c = host_inputs(x, h, W_ih, W_hh, b_ih, b_hh, W_proj, b_proj)
    nc = _get_nc(n_c)
    trace = bool(os.environ.get("KERNEL_TRACE"))
    res = run_bass_kernel_spmd(nc, in_maps, list(range(N_CORES)), trace=trace)
    LAST_RESULT = res

    n = n_c * N_CORES
    probs = np.empty((n, VOUT), np.float32)
    h_new = np.empty((n, HID), np.float32)
    for i in range(N_CORES):
        sl = slice(i * n_c, (i + 1) * n_c)
        # de-interleave [nsuper, 128, QRB, VOUT] -> rows (sc, rb, p) order
        probs[sl] = (
            res.results[i]["probs"].transpose(0, 2, 1, 3).reshape(n_c, VOUT)
        )
        h_new[sl] = res.results[i]["hnt"].T.astype(np.float32)
    return probs, h_new


# revision 21
# speedup vs baseline: 1.1171x; 1.0060x over previous
"""Trainium2 Bass kernel for a single-step vanilla RNN cell + projection + softmax.

Math (per row of the batch):
    h_new = tanh(x @ W_ih.T + b_ih + h @ W_hh.T + b_hh)
    probs = softmax(h_new @ W_proj.T + b_proj, axis=-1)
Returns (probs, h_new) as float32, matching the reference.

Strategy: pure data parallel over the batch dim n across 8 NeuronCores.
The host pre-transposes/casts activations to bf16 column-layout (features
on partitions, batch on the free dim), so the device runs a transpose-free
matmul pipeline:
  per 512-row chunk:
    h_newT[128,512]x2 (PSUM f32) = W_ihT_aug.T @ x_augT + W_hhT.T @ hT
       (bias rows folded into the augmented x/W_ih matrices)
    tanh -> bf16 SBUF tile (doubles as projection lhsT and DMA-out source)
    logits[128rows,20] = h_newT_slice.T @ W_projT    (row layout)
    softmax via exp (ACT) * exp(b_proj) const, row-sum, reciprocal, scale
Outputs: h_newT bf16 [256, n_c] (host transposes back to [n,256] f32),
probs f32 [n_c, 20] directly in natural layout.
"""

import os

import numpy as np
import ml_dtypes

import concourse.bass as bass
import concourse.mybir as mybir
import concourse.tile as tile
from concourse.bass_utils import run_bass_kernel_spmd

BF16 = mybir.dt.bfloat16
F32 = mybir.dt.float32
bf16 = ml_dtypes.bfloat16

N_CORES = 8
HID = 256
VIN = 40
VOUT = 20
CHUNK = 512
RB = CHUNK // 128  # row blocks of 128 per chunk

# Exposed for test harnesses: the BassKernelResults of the last run.
LAST_RESULT = None


SUPER = 4096  # rows per DMA superchunk (1 MB bf16 hidden-state transfers)
QRB = SUPER // 128  # row blocks of 128 per superchunk


def build_nc(n_c: int, dma: str = "sync", split_waits: bool = True) -> bass.Bass:
    assert n_c % SUPER == 0
    nsuper = n_c // SUPER
    nsub = SUPER // CHUNK
    nc = bass.Bass()
    dma_eng = {"sync": lambda: nc.sync, "gpsimd": lambda: nc.gpsimd}[dma]()
    # loads on the SP HWDGE ring, stores on the ACT HWDGE ring: stores wait on
    # ACT-produced data (same-engine program order, no blocking), so prefetch
    # loads are never queued behind a store that is waiting for compute.
    st_eng = nc.gpsimd

    xat_d = nc.dram_tensor("xat", [VIN + 1, n_c], BF16, kind="ExternalInput")
    ht_d = nc.dram_tensor("ht", [HID, n_c], BF16, kind="ExternalInput")
    w1_d = nc.dram_tensor("w1", [VIN + 1, HID], BF16, kind="ExternalInput")
    whh_d = nc.dram_tensor("whh", [HID, HID], BF16, kind="ExternalInput")
    wp_d = nc.dram_tensor("wp", [HID, VOUT], BF16, kind="ExternalInput")
    ebp_d = nc.dram_tensor("ebp", [128, RB, VOUT], F32, kind="ExternalInput")
    hnt_d = nc.dram_tensor("hnt", [HID, n_c], BF16, kind="ExternalOutput")
    # probs are stored in the SBUF-native interleaved layout (one contiguous
    # 2.5KB run per partition per superchunk); the host de-interleaves.
    pr_d = nc.dram_tensor(
        "probs", [n_c // SUPER, 128, QRB, VOUT], F32, kind="ExternalOutput"
    )

    with tile.TileContext(nc) as tc:
        with (
            tc.tile_pool(name="consts", bufs=1) as consts,
            tc.tile_pool(name="io", bufs=3) as io,
            tc.tile_pool(name="hsb", bufs=2) as hsb,
            tc.tile_pool(name="prs", bufs=2) as prs,
            tc.tile_pool(name="sfx", bufs=3) as sfx,
            tc.tile_pool(name="ps_h", bufs=4, space="PSUM") as ps_h,
            tc.tile_pool(name="ps_l", bufs=2, space="PSUM") as ps_l,
        ):
            w1 = consts.tile([VIN + 1, HID], BF16)
            nc.gpsimd.dma_start(out=w1[:], in_=w1_d[:])
            whh0 = consts.tile([128, HID], BF16, tag="whh0")
            whh1 = consts.tile([128, HID], BF16, tag="whh1")
            nc.gpsimd.dma_start(out=whh0[:], in_=whh_d[0:128, :])
            nc.gpsimd.dma_start(out=whh1[:], in_=whh_d[128:256, :])
            wp0 = consts.tile([128, VOUT], BF16, tag="wp0")
            wp1 = consts.tile([128, VOUT], BF16, tag="wp1")
            nc.gpsimd.dma_start(out=wp0[:], in_=wp_d[0:128, :])
            nc.gpsimd.dma_start(out=wp1[:], in_=wp_d[128:256, :])
            ebp = consts.tile([128, RB, VOUT], F32)
            nc.gpsimd.dma_start(out=ebp[:], in_=ebp_d[:])

            # Per-superchunk live tiles, keyed sc % 2 (pools have bufs>=2).
            tiles = {}

            def emit_loads(sc):
                ss = sc * SUPER
                xat = io.tile([VIN + 1, SUPER], BF16, tag="xat")
                ht0 = io.tile([128, SUPER], BF16, tag="ht0")
                ht1 = io.tile([128, SUPER], BF16, tag="ht1")
                if sc == 0:
                    # halve the first superchunk's loads so the first matmuls
                    # can start after 0.5MB instead of a full 1MB per tensor
                    h = SUPER // 2
                    for o in (0, h):
                        osl = slice(o, o + h)
                        dma_eng.dma_start(out=xat[:, osl], in_=xat_d[:, osl])
                        dma_eng.dma_start(out=ht0[:, osl], in_=ht_d[0:128, osl])
                        dma_eng.dma_start(out=ht1[:, osl], in_=ht_d[128:256, osl])
                else:
                    dma_eng.dma_start(out=xat[:], in_=xat_d[:, ss : ss + SUPER])
                    dma_eng.dma_start(out=ht0[:], in_=ht_d[0:128, ss : ss + SUPER])
                    dma_eng.dma_start(out=ht1[:], in_=ht_d[128:256, ss : ss + SUPER])
                hnt0 = hsb.tile([128, SUPER], BF16, tag="hnt0")
                hnt1 = hsb.tile([128, SUPER], BF16, tag="hnt1")
                pr = prs.tile([128, QRB, VOUT], F32, tag="pr")
                tiles[sc % 2] = (xat, ht0, ht1, hnt0, hnt1, pr)

            def emit_mm(sc, q):
                xat, ht0, ht1, _, _, _ = tiles[sc % 2]
                qs = q * CHUNK
                out = []
                for m in range(2):
                    ms = m * 128
                    ps = ps_h.tile([128, CHUNK], F32, tag="hnt_ps")
                    nc.tensor.matmul(
                        ps[:], w1[:, ms : ms + 128], xat[:, qs : qs + CHUNK],
                        start=True, stop=False,
                    )
                    nc.tensor.matmul(
                        ps[:], whh0[:, ms : ms + 128], ht0[:, qs : qs + CHUNK],
                        start=False, stop=False,
                    )
                    nc.tensor.matmul(
                        ps[:], whh1[:, ms : ms + 128], ht1[:, qs : qs + CHUNK],
                        start=False, stop=True,
                    )
                    out.append(ps)
                return out

            def emit_tanh(sc, q, pss):
                _, _, _, hnt0, hnt1, _ = tiles[sc % 2]
                qs = q * CHUNK
                for m, hnt in enumerate((hnt0, hnt1)):
                    nc.scalar.activation(
                        hnt[:, qs : qs + CHUNK], pss[m][:],
                        mybir.ActivationFunctionType.Tanh,
                    )

            def emit_proj_mm(sc, q):
                _, _, _, hnt0, hnt1, _ = tiles[sc % 2]
                qs = q * CHUNK
                lg = ps_l.tile([128, RB, VOUT], F32, tag="lg")
                for r in range(RB):
                    rs = qs + r * 128
                    nc.tensor.matmul(
                        lg[:, r, :], hnt0[:, rs : rs + 128], wp0[:],
                        start=True, stop=False,
                    )
                    nc.tensor.matmul(
                        lg[:, r, :], hnt1[:, rs : rs + 128], wp1[:],
                        start=False, stop=True,
                    )
                return lg

            def emit_softmax(sc, q, lg):
                _, _, _, _, _, pr = tiles[sc % 2]
                et = sfx.tile([128, RB, VOUT], F32, tag="et")
                nc.scalar.activation(et[:], lg[:], mybir.ActivationFunctionType.Exp)
                u = sfx.tile([128, RB, VOUT], F32, tag="u")
                nc.vector.tensor_mul(u[:], et[:], ebp[:])
                sm = sfx.tile([128, RB], F32, tag="sm")
                nc.vector.reduce_sum(sm[:], u[:], axis=mybir.AxisListType.X)
                rc = sfx.tile([128, RB], F32, tag="rc")
                nc.vector.reciprocal(rc[:], sm[:])
                for r in range(RB):
                    nc.vector.tensor_scalar_mul(
                        pr[:, q * RB + r, :], u[:, r, :], rc[:, r : r + 1]
                    )

            def emit_stores(sc):
                ss = sc * SUPER
                _, _, _, hnt0, hnt1, pr = tiles[sc % 2]
                st_eng.dma_start(out=hnt_d[0:128, ss : ss + SUPER], in_=hnt0[:])
                st_eng.dma_start(out=hnt_d[128:256, ss : ss + SUPER], in_=hnt1[:])
                st_eng.dma_start(out=pr_d[sc], in_=pr[:])

            # One-chunk software pipeline. Per iteration the PE stream is
            # [MM g][proj g-1] so the projection never waits on its own tanh,
            # and the ACT stream is [tanh g][exp g-1] so the next projection's
            # tanh is never queued behind softmax work.
            nchunks = nsuper * nsub
            prev = None
            for g in range(nchunks):
                sc, q = divmod(g, nsub)
                if q == 0:
                    emit_loads(sc)
                pss = emit_mm(sc, q)
                plg = emit_proj_mm(*prev) if prev is not None else None
                emit_tanh(sc, q, pss)
                if prev is not None:
                    emit_softmax(*prev, plg)
                    psc, pq = prev
                    if pq == nsub - 1:
                        emit_stores(psc)
                prev = (sc, q)
            plg = emit_proj_mm(*prev)
            emit_softmax(*prev, plg)
            emit_stores(prev[0])
    if split_waits:
        _split_excess_waits(nc)
    return nc


def _split_excess_waits(nc: bass.Bass) -> None:
    """Walrus allows only one sync-wait command per lowered instruction (the
    64B EVENTS struct has a single wait slot), but Tile's sem-assignment can
    attach 2+ waits (e.g. a data dep + the HWDGE queue-head wait) because the
    redundant-wait optimizer is disabled. Splitting the extra waits into
    standalone event-semaphore instructions immediately before, on the same
    engine stream, is semantically identical: the issuing sequencer blocks on
    them in program order before dispatching the instruction."""
    skip = {"InstCall", "InstUnconditionalBranch", "InstISA"}
    for fn in nc.m.functions:
        for bb in fn.blocks:
            out = []
            changed = False
            for ins in bb.instructions:
                si = ins.sync_info
                if (
                    si is not None
                    and len(si.on_wait) > 1
                    and type(ins).__name__ not in skip
                ):
                    waits = list(si.on_wait)
                    keep_idx = len(waits) - 1
                    for i, w in enumerate(waits):
                        if w.ant_name and w.ant_name.startswith(("DMAHW", "DMASW")):
                            keep_idx = i
                            break
                    for i, w in enumerate(waits):
                        if i == keep_idx:
                            continue
                        out.append(
                            mybir.InstEventSemaphore(
                                name=f"{ins.name}-xw{i}",
                                engine=ins.engine,
                                sync_info=mybir.SyncInfo(on_wait=[w], on_update=[]),
                                debug=ins.debug,
                            )
                        )
                    ins.sync_info = mybir.SyncInfo(
                        on_wait=[waits[keep_idx]], on_update=list(si.on_update)
                    )
                    changed = True
                out.append(ins)
            if changed:
                bb.instructions = out


_NC_CACHE = {}


def _get_nc(n_c: int) -> bass.Bass:
    if n_c not in _NC_CACHE:
        _NC_CACHE[n_c] = build_nc(n_c)
    return _NC_CACHE[n_c]


def host_inputs(x, h, W_ih, W_hh, b_ih, b_hh, W_proj, b_proj, n_cores=N_CORES):
    """Shard + relayout the full f32 inputs into per-core bf16 in_maps."""
    x = np.asarray(x, dtype=np.float32)
    h = np.asarray(h, dtype=np.float32)
    n = x.shape[0]
    n_c = n // n_cores

    w1 = np.empty((VIN + 1, HID), dtype=bf16)
    w1[:VIN] = np.asarray(W_ih, np.float32).T.astype(bf16)
    w1[VIN] = (
        np.asarray(b_ih, np.float32) + np.asarray(b_hh, np.float32)
    ).astype(bf16)
    whh = np.asarray(W_hh, np.float32).T.astype(bf16)
    wp = np.asarray(W_proj, np.float32).T.astype(bf16)
    ebp = np.ascontiguousarray(
        np.broadcast_to(
            np.exp(np.asarray(b_proj, np.float32))[None, None, :], (128, RB, VOUT)
        ),
        dtype=np.float32,
    )

    in_maps = []
    for i in range(n_cores):
        sl = slice(i * n_c, (i + 1) * n_c)
        xat = np.empty((VIN + 1, n_c), dtype=bf16)
        xat[:VIN] = x[sl].T.astype(bf16)
        xat[VIN] = 1.0
        ht = h[sl].T.astype(bf16)
        in_maps.append(
            {"xat": xat, "ht": ht, "w1": w1, "whh": whh, "wp": wp, "ebp": ebp}
        )
    return in_maps, n_c


def kernel(x, h, W_ih, W_hh, b_ih, b_hh, W_proj, b_proj):
    global LAST_RESULT
    in_maps, n_c = host_inputs(x, h, W_ih, W_hh, b_ih, b_hh, W_proj, b_proj)
    nc = _get_nc(n_c)
    trace = bool(os.environ.get("KERNEL_TRACE"))
    res = run_bass_kernel_spmd(nc, in_maps, list(range(N_CORES)), trace=trace)
    LAST_RESULT = res

    n = n_c * N_CORES
    probs = np.empty((n, VOUT), np.float32)
    h_new = np.empty((n, HID), np.float32)
    for i in range(N_CORES):
        sl = slice(i * n_c, (i + 1) * n_c)
        # de-interleave [nsuper, 128, QRB, VOUT] -> rows (sc, rb, p) order
        probs[sl] = (
            res.results[i]["probs"].transpose(0, 2, 1, 3).reshape(n_c, VOUT)
        )
        h_new[sl] = res.results[i]["hnt"].T.astype(np.float32)
    return probs, h_new
